# revision 46
# baseline (speedup 1.0000x reference)
"""Bidirectional attention kernel for Trainium2 (Bass/Tile), 8 NeuronCores.

Problem: B=32, L1=L2=1024, D=512 fp32.
  sim = v1 @ v2^T per batch; two masked softmaxes (axis 1 / axis 2);
  att_v1 = softmax_m(sim) @ v2 ; att_v2 = softmax_l(sim)^T @ v1; pad rows zeroed.

Sharding: data-parallel over batch, 4 batch slots per core, no cross-core comm.

Structure (v2 — dense host-packed pipeline):
- ~Half of each sequence is padding and contributes nothing to the visible
  output (pad exp weights are exp(-90) ~ 1e-39 and pad output rows are zeroed).
  The host compacts each batch's unmasked rows to n <= 128*T slots and also
  pre-transposes them, so the device runs a fully dense pipeline with no
  on-chip gathers or input transposes:
    v1T/v2T  [128, 4*LC]  fp32 (d-major)  -> f32r sim matmuls
    v1c/v2c  [128, T*512] bf16 (l-major)  -> attention matmul rhs
- Batches are sorted by compact tile count and striped across cores, so batch
  slot j is compiled with only the tiles its 8 batches need (T in {4,5} for
  this data). Slot shapes are derived from the actual masks at run time and
  the program is compiled (and cached) per shape signature.
- Softmax with a single global stabilizer exp(S - 90) (no row-max pass); row
  sums Z come free from the exp's fp32 accum_out.
- E is stored bf16 (weights only need ~8 mantissa bits; rel err ~2e-3 at the
  output): halves SBUF/PE-transpose cost. E^T tiles are PE-transposed (bf16,
  1.0 cyc/row), and the column sums W are computed by per-tile DVE reduces of
  E^T, replacing the ones-vector matmul pass entirely.
- sim f32r N-chunks are (384,256)-style splits: every chunk >= 256 wide keeps
  f32r at full PE rate (128-wide chunks run at 1/4 rate).
- Evictions fuse the 1/Z / 1/W scaling (ACT for att_v2, DVE for att_v1) and
  write bf16 into per-batch output strips; one dense store per output per
  batch, issued on the otherwise-idle Pool/SWDGE queue so store waits never
  block the load (SP) or compute (ACT/DVE) sequencers.
"""

import sys

if '/opt/trn_rl_repo' not in sys.path:
    sys.path.insert(0, '/opt/trn_rl_repo')

from contextlib import ExitStack

import numpy as np
import ml_dtypes

import concourse.bass as bass  # noqa: F401  (bass types referenced via tile APs)
import concourse.tile as tile
from concourse import bacc, mybir
from concourse import bass_utils

F32 = mybir.dt.float32
F32R = mybir.dt.float32r
BF16 = mybir.dt.bfloat16
BF = ml_dtypes.bfloat16
KSTAB = 90.0
ZEPS = 1e-30
VT_BF16 = True    # bf16 vT loads (sim in bf16): halves input DMA, rel err ~1e-2
ET_XBAR = False   # produce E^T via DMA XBAR transpose instead of PE+copies

B = 32
L = 1024
D = 512
PT = 128
NDT = D // PT        # 4 contraction d-chunks
N_CORES = 8
BPC = B // N_CORES   # batch slots per core


def _r(ap):
    return ap.bitcast(F32R)


def _nch(w):
    """Split width w (multiple of 128) into f32r-friendly chunks: <=512 wide
    and, wherever possible, >=256 wide (f32r matmuls run at 1/4 rate below
    256 output columns)."""
    out = []
    n0 = 0
    rem = w
    while rem > 0:
        if rem <= 512:
            c = rem
        elif rem <= 768:
            c = rem - 256
        else:
            c = 512
        out.append((n0, c))
        n0 += c
        rem -= c
    return out


def _build_batch(nc, pools, ident_bf, kbias, T1, T2, LC2, d,
                 first=False, last=False):
    LC1 = T1 * PT
    sb, st = pools["sb"], pools["st"]
    ps_sim, ps_att = pools["ps_sim"], pools["ps_att"]
    # m-chunk widths (last may be partial when LC2 is a 64-multiple)
    mws = [min(PT, LC2 - mc * PT) for mc in range(T2)]

    # ---- dense loads (host pre-compacted + pre-transposed) ----
    vt_dt = BF16 if VT_BF16 else F32R
    v1c = sb.tile([PT, T1 * D], BF16, tag="v1c")
    v2c = sb.tile([PT, T2 * D], BF16, tag="v2c")
    chunks = _nch(LC2)

    def _ld(dst, src):
        nc.sync.dma_start(dst, src if VT_BF16 else _r(src))

    if first:
        # combined head tensor: per d-chunk, [v1T_td | v2T_td] in ONE DMA —
        # halves the per-DMA HWDGE pacing during the cold start.
        W = LC1 + LC2
        vh = sb.tile([PT, NDT * W], vt_dt, tag="vh")
        for td in range(NDT):
            _ld(vh[:, td * W:(td + 1) * W], d["vh"][:, td * W:(td + 1) * W])
        v1Tf = lambda td: vh[:, td * W: td * W + LC1]
        v2Tf = lambda td: vh[:, td * W + LC1: (td + 1) * W]
    else:
        v1T = sb.tile([PT, NDT * LC1], vt_dt, tag="v1T")
        v2T = sb.tile([PT, NDT * LC2], vt_dt, tag="v2T")
        for td in range(NDT):
            _ld(v1T[:, td * LC1:(td + 1) * LC1], d["v1T"][:, td * LC1:(td + 1) * LC1])
            _ld(v2T[:, td * LC2:(td + 1) * LC2], d["v2T"][:, td * LC2:(td + 1) * LC2])
        v1Tf = lambda td: v1T[:, td * LC1:(td + 1) * LC1]
        v2Tf = lambda td: v2T[:, td * LC2:(td + 1) * LC2]
    nc.sync.dma_start(v1c[:], d["v1c"])
    nc.sync.dma_start(v2c[:], d["v2c"])

    # ---- similarity + exp -> E (bf16), row sums Z from accum_out ----
    # E^T tiles are produced by the DMA XBAR transpose engine (16x128 tiles),
    # issued from the ACT queue right after the exp that completes each
    # l-tile; the PE never touches them.
    E = sb.tile([PT, T1 * LC2], BF16, tag="E")
    ET = sb.tile([PT, T2 * LC1], BF16, tag="ET")
    ET_r = ET[:].rearrange("p (c l) -> p c l", c=T2)
    zparts = [st.tile([PT, T1], F32, tag=f"z{h}", name=f"zp{h}")
              for h in range(len(chunks))]
    # lt-outer: each l-tile of E completes early, so its E^T transpose and
    # W reduce pipeline through the sim phase. The first batch runs h-outer
    # to match its staged load waves.
    if first:
        pairs = [(h, lt) for h in range(len(chunks)) for lt in range(T1)]
    else:
        pairs = [(h, lt) for lt in range(T1) for h in range(len(chunks))]
    for h, lt in pairs:
        n0, nw = chunks[h]
        p_s = ps_sim.tile([PT, 512], F32, tag="psim")
        for td in range(NDT):
            nc.tensor.matmul(
                p_s[:, 0:nw],
                v1Tf(td)[:, lt * PT:(lt + 1) * PT],
                v2Tf(td)[:, n0:n0 + nw],
                start=(td == 0), stop=(td == NDT - 1))
        nc.scalar.activation(
            E[:, lt * LC2 + n0: lt * LC2 + n0 + nw], p_s[:, 0:nw],
            mybir.ActivationFunctionType.Exp,
            bias=kbias[:], scale=1.0,
            accum_out=zparts[h][:, lt:lt + 1])
        if not ET_XBAR and h == len(chunks) - 1:
            p_tr = pools["ps_tr"].tile([PT, T2 * PT], BF16, tag="ptr")
            for mc in range(T2):
                nc.tensor.transpose(
                    p_tr[0:mws[mc], mc * PT:(mc + 1) * PT],
                    E[:, lt * LC2 + mc * PT: lt * LC2 + mc * PT + mws[mc]],
                    ident_bf[:])
            src = p_tr[:].rearrange("p (c q) -> p c q", c=T2)
            dst = ET_r[:, :, lt * PT:(lt + 1) * PT]
            # batch 0's h-outer tail bunches exps on ACT; keep its copies off
            # the exp queue
            if first:
                nc.vector.tensor_copy(dst, src)
            else:
                nc.scalar.copy(dst, src)

    if ET_XBAR:
        # fire all E^T XBAR transposes once E is complete: no later writes to
        # E exist, so coarse DMA dependency tracking cannot serialize the sim.
        for lt in range(T1):
            nc.scalar.dma_start(ET_r[:, :, lt * PT:(lt + 1) * PT],
                                E[:, lt * LC2:(lt + 1) * LC2],
                                transpose=True)

    z2 = st.tile([PT, T1], F32, tag="z2")
    if len(zparts) == 1:
        nc.vector.tensor_scalar_add(z2[:], zparts[0][:], ZEPS)
    else:
        nc.vector.tensor_add(z2[:], zparts[0][:], zparts[1][:])
        for h in range(2, len(zparts)):
            nc.vector.tensor_add(z2[:], z2[:], zparts[h][:])
        nc.vector.tensor_scalar_add(z2[:], z2[:], ZEPS)
    rz2 = st.tile([PT, T1], F32, tag="rz2")
    nc.vector.reciprocal(rz2[:], z2[:])

    # ---- column sums W via per-l-tile DVE reduces of E^T ----
    wparts = []
    for lt in range(T1):
        wp = st.tile([PT, T2], F32, tag=f"wp{lt}", name=f"wp{lt}")
        nc.vector.tensor_reduce(wp[:], ET_r[:, :, lt * PT:(lt + 1) * PT],
                                axis=mybir.AxisListType.X,
                                op=mybir.AluOpType.add)
        wparts.append(wp)

    w2 = st.tile([PT, T2], F32, tag="w2")
    if len(wparts) == 1:
        nc.vector.tensor_scalar_add(w2[:], wparts[0][:], ZEPS)
    else:
        nc.vector.tensor_add(w2[:], wparts[0][:], wparts[1][:])
        for k in range(2, len(wparts)):
            nc.vector.tensor_add(w2[:], w2[:], wparts[k][:])
        nc.vector.tensor_scalar_add(w2[:], w2[:], ZEPS)
    rw2 = st.tile([PT, T2], F32, tag="rw2")
    nc.vector.reciprocal(rw2[:], w2[:])

    # ---- attention outputs, tile-interleaved ----
    o1g = sb.tile([PT, T1 * D], BF16, tag="o1g")
    o2g = sb.tile([PT, T2 * D], BF16, tag="o2g")
    for i in range(max(T1, T2)):
        if i < T1:   # att_v1 l-tile i: contraction over m, scale 1/Z (DVE)
            pa1 = ps_att.tile([PT, D], F32, tag="pa")
            # split the very last tile's output in half so the final
            # eviction + store (the kernel tail) are half-size
            halves = ((0, 256), (256, 256)) if (last and i == T1 - 1) \
                else ((0, D),)
            for d0, dw in halves:
                for mc in range(T2):
                    nc.tensor.matmul(
                        pa1[:, d0:d0 + dw],
                        ET[0:mws[mc], mc * LC1 + i * PT: mc * LC1 + (i + 1) * PT],
                        v2c[0:mws[mc], mc * D + d0: mc * D + d0 + dw],
                        start=(mc == 0), stop=(mc == T2 - 1))
                nc.vector.tensor_scalar_mul(
                    o1g[:, i * D + d0: i * D + d0 + dw],
                    pa1[:, d0:d0 + dw], rz2[:, i:i + 1])
                eng1 = nc.sync if (last and i == T1 - 1) else nc.gpsimd
                eng1.dma_start(d["o1c"][:, i * D + d0: i * D + d0 + dw],
                               o1g[:, i * D + d0: i * D + d0 + dw])
        if i < T2:   # att_v2 m-tile i: contraction over l, scale 1/W
            mw = mws[i]
            pa2 = ps_att.tile([PT, D], F32, tag="pa")
            for lc in range(T1):
                nc.tensor.matmul(
                    pa2[0:mw, :],
                    E[:, lc * LC2 + i * PT: lc * LC2 + i * PT + mw],
                    v1c[:, lc * D:(lc + 1) * D],
                    start=(lc == 0), stop=(lc == T1 - 1))
            if ET_XBAR:
                nc.vector.tensor_scalar_mul(
                    o2g[0:mw, i * D:(i + 1) * D], pa2[0:mw, :], rw2[0:mw, i:i + 1])
            else:
                nc.scalar.activation(
                    o2g[0:mw, i * D:(i + 1) * D], pa2[0:mw, :],
                    mybir.ActivationFunctionType.Copy,
                    bias=0.0, scale=rw2[0:mw, i:i + 1])
            eng2 = nc.sync if (last and i >= T2 - 2) else nc.gpsimd
            eng2.dma_start(d["o2c"][0:mw, i * D:(i + 1) * D],
                           o2g[0:mw, i * D:(i + 1) * D])


_CACHE = {}


def _get_compiled(shapes):
    key = tuple(shapes)
    if key in _CACHE:
        return _CACHE[key]

    nc = bacc.Bacc("TRN2", target_bir_lowering=False, debug=False,
                   enable_asserts=False, num_devices=N_CORES)

    vt_dt = BF16 if VT_BF16 else F32
    d_tensors = []
    for j, (T1, T2, LC2) in enumerate(shapes):
        t = {}
        if j == 0:
            t["vh"] = nc.dram_tensor(
                "vh_0", [PT, NDT * (T1 * PT + LC2)], vt_dt,
                kind="ExternalInput").ap()
        else:
            t["v1T"] = nc.dram_tensor(f"v1T_{j}", [PT, NDT * T1 * PT], vt_dt,
                                      kind="ExternalInput").ap()
            t["v2T"] = nc.dram_tensor(f"v2T_{j}", [PT, NDT * LC2], vt_dt,
                                      kind="ExternalInput").ap()
        t["v1c"] = nc.dram_tensor(f"v1c_{j}", [PT, T1 * D], BF16,
                                  kind="ExternalInput").ap()
        t["v2c"] = nc.dram_tensor(f"v2c_{j}", [PT, T2 * D], BF16,
                                  kind="ExternalInput").ap()
        t["o1c"] = nc.dram_tensor(f"o1c_{j}", [PT, T1 * D], BF16,
                                  kind="ExternalOutput").ap()
        t["o2c"] = nc.dram_tensor(f"o2c_{j}", [PT, T2 * D], BF16,
                                  kind="ExternalOutput").ap()
        d_tensors.append(t)
    id_d = None
    if not ET_XBAR:
        id_d = nc.dram_tensor("identb", [PT, PT], BF16, kind="ExternalInput").ap()

    with tile.TileContext(nc) as tc:
        with ExitStack() as ctx:
            pools = {
                "sb": ctx.enter_context(tc.tile_pool(name="sb", bufs=4)),
                "st": ctx.enter_context(tc.tile_pool(name="st", bufs=3)),
                "ps_sim": ctx.enter_context(
                    tc.tile_pool(name="ps_sim", bufs=4 if ET_XBAR else 3,
                                 space="PSUM")),
                "ps_att": ctx.enter_context(
                    tc.tile_pool(name="ps_att", bufs=4, space="PSUM")),
            }
            if not ET_XBAR:
                pools["ps_tr"] = ctx.enter_context(
                    tc.tile_pool(name="ps_tr", bufs=1, space="PSUM"))
            st = pools["st"]
            ident_bf = None
            if not ET_XBAR:
                ident_bf = st.tile([PT, PT], BF16, tag="identb", bufs=1)
                nc.sync.dma_start(ident_bf[:], id_d)
            kbias = st.tile([PT, 1], F32, tag="kbias", bufs=1)
            nc.vector.memset(kbias[:], -KSTAB)
            for j, (T1, T2, LC2) in enumerate(shapes):
                _build_batch(nc, pools, ident_bf, kbias,
                             T1, T2, LC2, d_tensors[j],
                             first=(j == 0), last=(j == len(shapes) - 1))

    nc.compile()
    _CACHE[key] = nc
    return nc


def _plan(v1_mask, v2_mask):
    """Sort batches by compact tile count, stripe across cores; slot shape =
    componentwise max over its 8 batches."""
    n1 = (~v1_mask).sum(axis=1).astype(int)
    n2 = (~v2_mask).sum(axis=1).astype(int)
    t1 = np.maximum(1, -(-n1 // PT))
    t2 = np.maximum(1, -(-n2 // PT))
    order = sorted(range(B), key=lambda b: (-t1[b], -t2[b], -(n1[b] + n2[b]), b))
    assign = [[0] * BPC for _ in range(N_CORES)]
    shapes = []
    for j in range(BPC):
        grp = order[j * N_CORES:(j + 1) * N_CORES]
        for k, b in enumerate(grp):
            assign[k][j] = b
        mx2 = int(max(n2[b] for b in grp))
        shapes.append((int(max(t1[b] for b in grp)),
                       int(max(t2[b] for b in grp)),
                       max(64, -(-mx2 // 64) * 64)))
    return assign, tuple(shapes)


def _pack(vb, idx, T, lcx=None):
    """[L, D] fp32 + keep-indices -> (vT [128, 4*lcx] f32, vc [128, T*512] bf16).

    vT[p, td*lcx + l] = v[idx[l], td*128 + p]   (zero-padded slots)
    vc[p, c*512 + d] = v[idx[c*128 + p], d]
    """
    LC = T * PT
    if lcx is None:
        lcx = LC
    a = np.zeros((LC, D), np.float32)
    a[:len(idx)] = vb[idx]
    vT = np.ascontiguousarray(
        a[:lcx].T.reshape(NDT, PT, lcx).transpose(1, 0, 2).reshape(PT, NDT * lcx))
    if VT_BF16:
        vT = vT.astype(BF)
    vc = np.ascontiguousarray(
        a.reshape(T, PT, D).transpose(1, 0, 2).reshape(PT, T * D)).astype(BF)
    return vT, vc


def run_on_device(v1, v1_mask, v2, v2_mask, trace=False):
    v1 = np.asarray(v1)
    v2 = np.asarray(v2)
    v1_mask = np.asarray(v1_mask).astype(bool)
    v2_mask = np.asarray(v2_mask).astype(bool)

    assign, shapes = _plan(v1_mask, v2_mask)
    nc = _get_compiled(shapes)

    idx1s, idx2s = {}, {}
    in_maps = []
    for core in range(N_CORES):
        m = {} if ET_XBAR else {"identb": np.eye(PT, dtype=BF)}
        for j in range(BPC):
            b = assign[core][j]
            idx1 = np.where(~v1_mask[b])[0]
            idx2 = np.where(~v2_mask[b])[0]
            idx1s[b], idx2s[b] = idx1, idx2
            T1, T2, LC2 = shapes[j]
            p1T, m[f"v1c_{j}"] = _pack(v1[b], idx1, T1)
            p2T, m[f"v2c_{j}"] = _pack(v2[b], idx2, T2, LC2)
            if j == 0:
                LC1 = T1 * PT
                m["vh_0"] = np.concatenate(
                    [np.concatenate(
                        [p1T[:, td * LC1:(td + 1) * LC1],
                         p2T[:, td * LC2:(td + 1) * LC2]], axis=1)
                     for td in range(NDT)], axis=1)
            else:
                m[f"v1T_{j}"], m[f"v2T_{j}"] = p1T, p2T
        in_maps.append(m)

    res = bass_utils.run_bass_kernel_spmd(
        nc, in_maps, core_ids=list(range(N_CORES)), trace=trace)

    att1 = np.zeros((B, L, D), np.float32)
    att2 = np.zeros((B, L, D), np.float32)
    for core in range(N_CORES):
        for j in range(BPC):
            b = assign[core][j]
            T1, T2, _ = shapes[j]
            o1 = np.asarray(res.results[core][f"o1c_{j}"]).astype(np.float32)
            o2 = np.asarray(res.results[core][f"o2c_{j}"]).astype(np.float32)
            r1 = o1.reshape(PT, T1, D).transpose(1, 0, 2).reshape(T1 * PT, D)
            r2 = o2.reshape(PT, T2, D).transpose(1, 0, 2).reshape(T2 * PT, D)
            att1[b][idx1s[b]] = r1[:len(idx1s[b])]
            att2[b][idx2s[b]] = r2[:len(idx2s[b])]
    return (att1, att2), res


def kernel(v1, v1_mask, v2, v2_mask):
    (att_v1, att_v2), _ = run_on_device(v1, v1_mask, v2, v2_mask)
    return (att_v1, att_v2)
